# revision 23
# baseline (speedup 1.0000x reference)
"""AttnBlock (GroupNorm + single-head self-attention + residual) on 8 NeuronCores.

Sharding: data-parallel over B (4 batches) x sequence-parallel over query
rows (2 halves of H*W=4096) = 8 shards, one per core.  Each core loads its
batch's full x[b] as [C=128, HW=4096] fp16 (channels on partitions), with
the spatial columns rotated so the core's query half is cols [0:2048).

GroupNorm is folded into the projections (h = A*x + B per channel), so only
the [128,128] weight scalings depend on the statistics.  Biases are kept
positive (b16 = +B via a negated-affA stt) so PSUM evacuations are single
fused adds on either VectorE (tensor_scalar) or ScalarE (Identity+bias).

Query halves are processed SEQUENTIALLY: PSUM holds the st double-buffer
(4 banks) + one half's PV accumulator (2 banks) + a 2-bank scratch rotation,
so the k/v/q projections overlap the first half's attention loop and half
0's softmax epilogue overlaps half 1's loop.  Scores for iteration i+1 are
emitted between exp(i) and PV(i) so the PE queue order guarantees a
back-to-back exp stream, including at the half boundary.

Input x streams over 3 DMA queues + a vector-issued chunk; GroupNorm sums
use per-chunk accumulator outputs (Sum-x on VectorE tensor_scalar, Sum-x2
split ScalarE Square / VectorE stt) so stats finish ~with the DMA.  Softmax
denominators: e-tiles accumulate on VectorE (blocks 0-23 -> esA, 24-31 ->
esB); one-hot-column matmuls column-sum esA in-loop and esB at the end into
an [8,128] PSUM tile, reciprocal_approx_fast + cast, then 8 selector
matmuls broadcast 1/sum back to [C,1024].  The selector/one-hot constants
are generated on-device by GpSimd memsets (keeps the weight DMA small).
exp runs on ScalarE with fused bias (-8; scores bounded for this problem
family, keeps exp within fp16 range).  rstd uses exp(-0.5*ln(var+eps)) and
the activation-table chooser is pinned to the natural_log_exp set, so the
kernel performs exactly one ACT_TABLE_LOAD, hidden behind the input DMA.
All matmuls run fp16.
"""

import numpy as np

C = 128
HW = 4096
NQ = 2048  # queries per core
HALF = 1024  # queries per sequential half
JB = 32  # key blocks of 128
ESPLIT = 24  # e-sum accumulator split: esA = blocks [0,24), esB = [24,32)
EXP_BIAS = -8.0
EPS = 1e-5
N_CORES = 8
N_WARM = 12  # dummy matmuls to lift the PE HAM clock-gate at start
RSUM = 1.0 / (4 * HW)  # per-group mean normalizer (4 channels x HW)

# wmid (f16) column offsets; cols [_FP32:_FP32+4) hold [nw|nb] as hi/lo
# f16 pairs (value = hi + lo, exact to ~22 bits once re-added on device)
_GMAP, _GMAPT, _FP32 = 0, 32, 160
_WMID_W = 164
# wbig (f16) column offsets
_WQ, _WK, _WV, _WO = 0, 128, 256, 384
_WBIG_W = 512

_NC = None


def _pin_activation_tables():
    """Restrict the table-load chooser to natural_log_exp_and_others so the
    kernel's ACT stream (ln/exp/square/copy/identity) needs one table load."""
    from concourse.hw_specs import get_activation_tables
    tabs = get_activation_tables("gen3")
    for name in list(tabs.keys()):
        if name != "natural_log_exp_and_others":
            tabs[name] = set()


def _build_program():
    import concourse.bacc as bacc
    import concourse.tile as tile
    from concourse import mybir

    f32 = mybir.dt.float32
    f16 = mybir.dt.float16
    AF = mybir.ActivationFunctionType
    OP = mybir.AluOpType

    nc = bacc.Bacc("TRN2", target_bir_lowering=False, debug=False,
                   num_devices=N_CORES)
    try:
        _pin_activation_tables()
    except Exception:
        pass

    x_d = nc.declare_dram_parameter("x", [C, HW], f16, isOutput=False)
    wmid_d = nc.declare_dram_parameter("wmid", [C, _WMID_W], f16,
                                       isOutput=False)
    wbig_d = nc.declare_dram_parameter("wbig", [C, _WBIG_W], f16,
                                       isOutput=False)
    y_d = nc.declare_dram_parameter("y", [C, NQ], f16, isOutput=True)

    with tile.TileContext(nc) as tc:
        consts = tc.alloc_tile_pool(name="consts", bufs=1)
        big = tc.alloc_tile_pool(name="big", bufs=1)
        work = tc.alloc_tile_pool(name="work", bufs=4)
        epool = tc.alloc_tile_pool(name="epool", bufs=3)
        esums = tc.alloc_tile_pool(name="esums", bufs=2)
        opool = tc.alloc_tile_pool(name="opool", bufs=2)
        # PSUM: st = 2 x [128,1024]f32 (4 banks), ot = 1 x (2 banks),
        # pj = 2 x [128,512]f32 (2 banks) scratch rotation
        pst = tc.alloc_tile_pool(name="pst", bufs=2, space="PSUM")
        pot = tc.alloc_tile_pool(name="pot", bufs=1, space="PSUM")
        pj = tc.alloc_tile_pool(name="pj", bufs=2, space="PSUM")

        def pjt(nm, shape=(C, 512)):
            return pj.tile(list(shape), f32, tag="x", name=nm,
                           padded_shape=[C, 512])

        # ---- PE warmup: dummy matmul burst covering the DMA window so the
        # HAM clock-gate reaches K=8/8 before the stats-chain matmuls.
        wz = consts.tile([C, 512], f16)
        nc.vector.memset(wz, 0.0)
        warm_ps = pst.tile([C, 512], f32, tag="st", name="warm",
                           padded_shape=[C, HALF])
        for _ in range(N_WARM):
            nc.tensor.matmul(out=warm_ps, lhsT=wz[:, 0:C], rhs=wz)

        # ---- input DMAs: x in 8 x 512-col chunks over sync/scalar/gpsimd/
        # vector queues; weights split small-first so gmap lands early.
        x16 = big.tile([C, HW], f16)

        def xdma(eng, ch):
            eng.dma_start(out=x16[:, ch * 512:(ch + 1) * 512],
                          in_=x_d.ap()[:, ch * 512:(ch + 1) * 512])

        wmid_sb = consts.tile([C, _WMID_W], f16)
        wbig_sb = consts.tile([C, _WBIG_W], f16)
        xdma(nc.sync, 0)
        xdma(nc.scalar, 1)
        nc.gpsimd.dma_start(out=wmid_sb, in_=wmid_d.ap())
        xdma(nc.sync, 3)
        xdma(nc.scalar, 4)
        nc.gpsimd.dma_start(out=x16[:, 2 * 512:3 * 512],
                            in_=x_d.ap()[:, 2 * 512:3 * 512])
        xdma(nc.sync, 6)
        xdma(nc.scalar, 7)
        nc.gpsimd.dma_start(out=x16[:, 5 * 512:6 * 512],
                            in_=x_d.ap()[:, 5 * 512:6 * 512])
        nc.gpsimd.dma_start(out=wbig_sb, in_=wbig_d.ap())

        wq_sb = wbig_sb[:, _WQ:_WQ + C]
        wk_sb = wbig_sb[:, _WK:_WK + C]
        wv_sb = wbig_sb[:, _WV:_WV + C]
        wo_sb = wbig_sb[:, _WO:_WO + C]
        gmap_sb = wmid_sb[:, _GMAP:_GMAP + 32]
        gmapt_sb = wmid_sb[0:32, _GMAPT:_GMAPT + C]
        nwnb = work.tile([C, 2], f32)
        nc.vector.tensor_add(out=nwnb, in0=wmid_sb[:, _FP32:_FP32 + 2],
                             in1=wmid_sb[:, _FP32 + 2:_FP32 + 4])
        nw_sb = nwnb[:, 0:1]
        nb_sb = nwnb[:, 1:2]
        eps_sb = consts.tile([32, 1], f32)
        nc.vector.memset(eps_sb, EPS)
        ebias_sb = consts.tile([C, 1], f32)
        nc.vector.memset(ebias_sb, EXP_BIAS)

        # selector constants generated on-device (GpSimd is idle after its
        # DMA issues; keeps them out of the weight DMA).  scol8 columns are
        # plain memsets; sel8 rows (row k of block k all-ones) cannot be
        # memset per-partition, so they come from K=1 one-hot x ones
        # matmuls on the idle PE, evacuated on the idle ACT.
        sel8_sb = consts.tile([8, 8 * C], f16)
        scol8_sb = consts.tile([C, 64], f16)
        nc.gpsimd.memset(scol8_sb, 0.0)
        for k in range(8):
            nc.gpsimd.memset(scol8_sb[:, k * 8 + k:k * 8 + k + 1], 1.0)
        oh8 = consts.tile([1, 64], f16)
        nc.gpsimd.memset(oh8, 0.0)
        for k in range(8):
            nc.gpsimd.memset(oh8[0:1, k * 8 + k:k * 8 + k + 1], 1.0)
        ones128 = consts.tile([1, C], f16)
        nc.gpsimd.memset(ones128, 1.0)

        # ---- GroupNorm sums: per-chunk accumulator outputs.  Sum-x on DVE
        # (tensor_scalar accum); Sum-x2 on ACT (Square accum) for the early
        # chunks and DVE (stt accum) for the last three.
        sxs = work.tile([C, 8], f32)
        sqs = work.tile([C, 8], f32)
        for ch in range(8):
            xc = x16[:, ch * 512:(ch + 1) * 512]
            dumv = work.tile([C, 512], f16, tag="dumv", name=f"dv{ch}",
                             bufs=2)
            nc.vector.tensor_scalar(out=dumv, in0=xc, scalar1=1.0,
                                    scalar2=None, op0=OP.mult, op1=OP.add,
                                    accum_out=sxs[:, ch:ch + 1])
            if ch < 5:
                duma = work.tile([C, 512], f16, tag="duma", name=f"da{ch}",
                                 bufs=2)
                nc.scalar.activation(out=duma, in_=xc, func=AF.Square,
                                     accum_out=sqs[:, ch:ch + 1])
            else:
                dums = work.tile([C, 512], f16, tag="dums", name=f"ds{ch}",
                                 bufs=2)
                nc.vector.scalar_tensor_tensor(
                    out=dums, in0=xc, scalar=1.0, in1=xc,
                    op0=OP.mult, op1=OP.mult,
                    accum_out=sqs[:, ch:ch + 1])
        sumx = work.tile([C, 1], f32)
        nc.vector.tensor_reduce(out=sumx, in_=sxs, op=OP.add,
                                axis=mybir.AxisListType.X)
        sumq = work.tile([C, 1], f32)
        nc.vector.tensor_reduce(out=sumq, in_=sqs, op=OP.add,
                                axis=mybir.AxisListType.X)
        spack = work.tile([C, 2], f16)
        nc.vector.tensor_copy(out=spack[:, 0:1], in_=sumx)
        nc.vector.tensor_copy(out=spack[:, 1:2], in_=sumq)
        gsum = pjt("gsum", (32, 2))
        nc.tensor.matmul(out=gsum, lhsT=gmap_sb, rhs=spack)
        # group mean (f16, into gpack) and -group var
        gpack = work.tile([32, 2], f16)
        nc.vector.tensor_scalar_mul(out=gpack[:, 0:1], in0=gsum[:, 0:1],
                                    scalar1=RSUM)
        ge2 = work.tile([32, 1], f32)
        nc.vector.tensor_scalar_mul(out=ge2, in0=gsum[:, 1:2], scalar1=RSUM)
        negvar = work.tile([32, 1], f32)
        nc.vector.scalar_tensor_tensor(
            out=negvar, in0=gpack[:, 0:1], scalar=gpack[:, 0:1], in1=ge2,
            op0=OP.mult, op1=OP.subtract)  # mean^2 - E[x^2] = -var
        # rstd = exp(-0.5 * ln(var+eps)); ln's scale=-1 flips -var
        gln = work.tile([32, 1], f32)
        nc.scalar.activation(out=gln, in_=negvar, func=AF.Ln, bias=eps_sb,
                             scale=-1.0)
        nc.scalar.activation(out=gpack[:, 1:2], in_=gln, func=AF.Exp,
                             scale=-0.5)
        cstat = pjt("cstat", (C, 2))
        nc.tensor.matmul(out=cstat, lhsT=gmapt_sb, rhs=gpack)
        affA = work.tile([C, 1], f32)
        nc.vector.tensor_mul(out=affA, in0=cstat[:, 1:2], in1=nw_sb)
        negA = work.tile([C, 1], f32)
        nc.vector.tensor_scalar_mul(out=negA, in0=affA, scalar1=-1.0)
        b16 = work.tile([C, 1], f16)  # = +B (affine shift)
        nc.vector.scalar_tensor_tensor(
            out=b16, in0=cstat[:, 0:1], scalar=negA, in1=nb_sb,
            op0=OP.mult, op1=OP.add)

        # ---- fold affine scale into projection weights; biases via tiny MMs
        wqa = consts.tile([C, C], f16)
        nc.vector.tensor_scalar_mul(out=wqa, in0=wq_sb, scalar1=affA)
        wka = consts.tile([C, C], f16)
        nc.vector.tensor_scalar_mul(out=wka, in0=wk_sb, scalar1=affA)
        wva = consts.tile([C, C], f16)
        nc.vector.tensor_scalar_mul(out=wva, in0=wv_sb, scalar1=affA)
        pb = pjt("pb", (C, 3))
        nc.tensor.matmul(out=pb[:, 0:1], lhsT=wq_sb, rhs=b16)
        nc.tensor.matmul(out=pb[:, 1:2], lhsT=wk_sb, rhs=b16)
        nc.tensor.matmul(out=pb[:, 2:3], lhsT=wv_sb, rhs=b16)
        qb = work.tile([C, 1], f32)
        nc.vector.tensor_copy(out=qb, in_=pb[:, 0:1])
        kb = work.tile([C, 1], f32)
        nc.vector.tensor_copy(out=kb, in_=pb[:, 1:2])
        vb16 = work.tile([C, 1], f16)
        nc.vector.tensor_copy(out=vb16, in_=pb[:, 2:3])
        pob = pjt("pob", (C, 1))
        nc.tensor.matmul(out=pob, lhsT=wo_sb, rhs=vb16)
        ob = work.tile([C, 1], f32)
        nc.vector.tensor_copy(out=ob, in_=pob)

        # sel8 row-selector blocks via K=1 outer products (PE idle here);
        # evacuated on ACT, which is idle between the rstd exp and the
        # first projection evacuations
        sel_ps = [pjt("selA", (8, 512)), pjt("selB", (8, 512))]
        for k in range(8):
            nc.tensor.matmul(out=sel_ps[k // 4][0:8,
                                                (k % 4) * C:(k % 4 + 1) * C],
                             lhsT=oh8[0:1, k * 8:(k + 1) * 8], rhs=ones128)
        nc.scalar.activation(out=sel8_sb[:, 0:512], in_=sel_ps[0],
                             func=AF.Copy)
        nc.scalar.activation(out=sel8_sb[:, 512:1024], in_=sel_ps[1],
                             func=AF.Copy)

        qT = big.tile([C, NQ], f16)
        kT = big.tile([C, HW], f16)
        v_sb = big.tile([C, HW], f16)  # col block jb holds V0[j, c] rows
        y_sb = big.tile([C, NQ], f16)

        def proj_q(ch, on_act=False):  # 512-col chunk of qT
            ps = pjt(f"qp{ch}")
            nc.tensor.matmul(out=ps, lhsT=wqa,
                             rhs=x16[:, ch * 512:(ch + 1) * 512])
            dst = qT[:, ch * 512:(ch + 1) * 512]
            if on_act:
                nc.scalar.activation(out=dst, in_=ps, func=AF.Identity,
                                     bias=qb)
            else:
                nc.vector.tensor_scalar(out=dst, in0=ps, scalar1=qb,
                                        scalar2=None, op0=OP.add)

        def proj_k(ch):
            ps = pjt(f"kp{ch}")
            nc.tensor.matmul(out=ps, lhsT=wka,
                             rhs=x16[:, ch * 512:(ch + 1) * 512])
            nc.vector.tensor_scalar(
                out=kT[:, ch * 512:(ch + 1) * 512], in0=ps, scalar1=kb,
                scalar2=None, op0=OP.add)

        def proj_v(ch, on_act=False):
            ps = pjt(f"vp{ch}")
            for k in range(4):
                jb = ch * 4 + k
                nc.tensor.matmul(out=ps[:, k * 128:(k + 1) * 128],
                                 lhsT=x16[:, jb * 128:(jb + 1) * 128],
                                 rhs=wva)
            dst = v_sb[:, ch * 512:(ch + 1) * 512]
            if on_act:
                nc.scalar.activation(out=dst, in_=ps, func=AF.Copy)
            else:
                nc.vector.tensor_copy(out=dst, in_=ps)

        # ---- minimal pre-loop projections: q half0, k/v blocks 0..3
        # (evacuations split across DVE and the still-idle ACT)
        proj_q(0)
        proj_q(1, on_act=True)
        proj_k(0)
        proj_v(0, on_act=True)

        # remaining projection jobs, one per early iteration of half 0;
        # k/v chunk c covers key blocks 4c..4c+3, first used at iteration 4c
        jobs = [lambda: proj_k(1), lambda: proj_v(1),
                lambda: proj_k(2), lambda: proj_v(2), lambda: proj_q(2),
                lambda: proj_k(3), lambda: proj_v(3),
                lambda: proj_k(4), lambda: proj_v(4), lambda: proj_q(3),
                lambda: proj_k(5), lambda: proj_v(5),
                lambda: proj_k(6), lambda: proj_v(6),
                lambda: proj_k(7), lambda: proj_v(7)]

        def epilogue_pieces(half, oT, esA, esB, s8, post_loop):
            """List of thunks, each emitting one chunk of the softmax/output
            epilogue for `half`.  post_loop=True uses the then-idle ScalarE
            for the PSUM evacuations; otherwise (overlapping the other
            half's loop, where ACT is exp-bound) everything sits on DVE."""
            oc16 = opool.tile([C, HALF], f16, tag="oc", name=f"oc{half}")
            r8f = opool.tile([8, C], f32, tag="r8f", name=f"r8f_{half}")
            r8 = opool.tile([8, C], f16, tag="r8", name=f"r8_{half}")
            rbc = [None, None]
            rc = opool.tile([C, HALF], f16, tag="rc", name=f"rc{half}")
            onrm = opool.tile([C, HALF], f16, tag="on", name=f"on{half}")
            i0 = half * HALF

            def ev_oc(k):
                def f():
                    dst = oc16[:, k * 512:(k + 1) * 512]
                    src = oT[:, k * 512:(k + 1) * 512]
                    if post_loop:
                        nc.scalar.activation(out=dst, in_=src, func=AF.Copy)
                    else:
                        nc.vector.tensor_copy(out=dst, in_=src)
                return f

            def mk_s8b(k):
                def f():
                    for ib in range(k * 4, k * 4 + 4):
                        nc.tensor.matmul(
                            out=s8[0],
                            lhsT=scol8_sb[:, ib * 8:(ib + 1) * 8],
                            rhs=esB[:, ib * 128:(ib + 1) * 128],
                            start=False, stop=(ib == 7))
                return f

            def mk_recip():
                nc.vector.reciprocal_approx_fast(out=r8f, in_=s8[0])

            def mk_cast():
                nc.vector.tensor_copy(out=r8, in_=r8f)

            def mk_rbc(k):
                def f():
                    rbc[k] = pjt(f"rbc{k}_{half}")
                    for k2 in range(4):
                        sel = k * 4 + k2
                        nc.tensor.matmul(
                            out=rbc[k][:, k2 * 128:(k2 + 1) * 128],
                            lhsT=sel8_sb[:, sel * C:(sel + 1) * C], rhs=r8)
                return f

            def ev_rc(k):
                def f():
                    dst = rc[:, k * 512:(k + 1) * 512]
                    if post_loop:
                        nc.scalar.activation(out=dst, in_=rbc[k],
                                             func=AF.Copy)
                    else:
                        nc.vector.tensor_copy(out=dst, in_=rbc[k])
                return f

            def mk_onrm(k):
                def f():
                    nc.vector.tensor_mul(
                        out=onrm[:, k * 512:(k + 1) * 512],
                        in0=oc16[:, k * 512:(k + 1) * 512],
                        in1=rc[:, k * 512:(k + 1) * 512])
                return f

            def mk_out(k):
                def f():
                    op = pjt(f"op{k}_{half}")
                    nc.tensor.matmul(out=op, lhsT=wo_sb,
                                     rhs=onrm[:, k * 512:(k + 1) * 512])
                    c0 = i0 + k * 512
                    nc.vector.scalar_tensor_tensor(
                        out=y_sb[:, c0:c0 + 512], in0=op, scalar=ob,
                        in1=x16[:, c0:c0 + 512], op0=OP.add, op1=OP.add)
                return f

            def mk_dma():
                nc.sync.dma_start(out=y_d.ap()[:, i0:i0 + HALF],
                                  in_=y_sb[:, i0:i0 + HALF])

            return [ev_oc(0), ev_oc(1), mk_s8b(0), mk_s8b(1),
                    mk_recip, mk_cast,
                    mk_rbc(0), ev_rc(0), mk_rbc(1), ev_rc(1),
                    mk_onrm(0), mk_onrm(1), mk_out(0), mk_out(1), mk_dma]

        # ---- main attention loop, one query half at a time, with scores
        # for iteration i+1 emitted between exp(i) and PV(i)
        iters = [(h, j) for h in (0, 1) for j in range(JB)]
        st_tiles = {}

        def emit_scores(h, j):
            st = pst.tile([C, HALF], f32, tag="st", name=f"st{h}_{j}")
            for k in range(2):
                nc.tensor.matmul(
                    out=st[:, k * 512:(k + 1) * 512],
                    lhsT=kT[:, j * 128:(j + 1) * 128],
                    rhs=qT[:, h * HALF + k * 512:h * HALF + (k + 1) * 512])
            st_tiles[(h, j)] = st

        pending = []
        oT = esA = esB = s8ref = None
        emit_scores(0, 0)
        for idx, (half, jb) in enumerate(iters):
            if jb == 0:
                oT = pot.tile([C, HALF], f32, tag="ot", name=f"oT{half}")
                esA = esums.tile([C, HALF], f16, tag="esA", name=f"esA{half}")
                esB = esums.tile([C, HALF], f16, tag="esB", name=f"esB{half}")
                s8ref = [None]
            st = st_tiles.pop((half, jb))
            e_t = epool.tile([C, HALF], f16, tag="e", name=f"e{half}_{jb}")
            nc.scalar.activation(out=e_t, in_=st, func=AF.Exp, bias=ebias_sb)
            if idx + 1 < len(iters):
                emit_scores(*iters[idx + 1])
            for k in range(2):
                nc.tensor.matmul(
                    out=oT[:, k * 512:(k + 1) * 512],
                    lhsT=v_sb[:, jb * 128:(jb + 1) * 128],
                    rhs=e_t[:, k * 512:(k + 1) * 512],
                    start=(jb == 0), stop=(jb == JB - 1))
            if jb == 0:
                nc.vector.tensor_copy(out=esA, in_=e_t)
            elif jb < ESPLIT:
                nc.vector.tensor_add(out=esA, in0=esA, in1=e_t)
            elif jb == ESPLIT:
                nc.vector.tensor_copy(out=esB, in_=e_t)
            else:
                nc.vector.tensor_add(out=esB, in0=esB, in1=e_t)
            # in-loop extra work: projections (half 0), the other half's
            # epilogue pieces (half 1, every other iteration), and this
            # half's esA column sums once esA is final (jb 25/27)
            if half == 0 and jb < len(jobs):
                jobs[jb]()
            if half == 1 and jb % 2 == 1 and pending:
                pending.pop(0)()
            if jb == ESPLIT + 1 or jb == ESPLIT + 3:
                k = (jb - ESPLIT - 1) // 2
                if k == 0:
                    s8ref[0] = pjt(f"s8_{half}", (8, C))
                for ib in range(k * 4, k * 4 + 4):
                    nc.tensor.matmul(
                        out=s8ref[0],
                        lhsT=scol8_sb[:, ib * 8:(ib + 1) * 8],
                        rhs=esA[:, ib * 128:(ib + 1) * 128],
                        start=(ib == 0), stop=False)
            if jb == JB - 1:
                if half == 0:
                    pending = epilogue_pieces(0, oT, esA, esB, s8ref,
                                              post_loop=False)
                    # oT0 evacuation is emitted at the half boundary so the
                    # first half-1 PV (which reuses oT's banks) never waits
                    # on not-yet-emitted copies
                    pending.pop(0)()
                    pending.pop(0)()
                else:
                    while pending:
                        pending.pop(0)()
                    for piece in epilogue_pieces(1, oT, esA, esB, s8ref,
                                                 post_loop=True):
                        piece()

        for p in (pj, pot, pst, opool, esums, epool, work, big, consts):
            p.release()

    nc.compile()
    return nc


def _get_nc():
    global _NC
    if _NC is None:
        _NC = _build_program()
    return _NC


def _make_packs(inputs):
    wq = (np.asarray(inputs["Wq"], dtype=np.float32) * (C ** -0.5)).astype(np.float16)
    wk = np.asarray(inputs["Wk"], dtype=np.float32).astype(np.float16)
    wv = np.asarray(inputs["Wv"], dtype=np.float32).astype(np.float16)
    wo = np.asarray(inputs["Wo"], dtype=np.float32).astype(np.float16)
    gmap = np.zeros((C, 32), np.float16)
    for c in range(C):
        gmap[c, c // 4] = 1.0
    wmid = np.zeros((C, _WMID_W), np.float16)
    wmid[:, _GMAP:_GMAP + 32] = gmap
    wmid[0:32, _GMAPT:_GMAPT + C] = gmap.T
    fp = np.empty((C, 2), np.float32)
    fp[:, 0] = np.asarray(inputs["norm_w"], dtype=np.float32)
    fp[:, 1] = np.asarray(inputs["norm_b"], dtype=np.float32)
    hi = fp.astype(np.float16)
    lo = (fp - hi.astype(np.float32)).astype(np.float16)
    wmid[:, _FP32:_FP32 + 2] = hi
    wmid[:, _FP32 + 2:_FP32 + 4] = lo
    wbig = np.zeros((C, _WBIG_W), np.float16)
    wbig[:, _WQ:_WQ + C] = wq
    wbig[:, _WK:_WK + C] = wk
    wbig[:, _WV:_WV + C] = wv
    wbig[:, _WO:_WO + C] = wo
    return wmid, wbig


def _make_in_maps(inputs):
    x = np.asarray(inputs["x"], dtype=np.float32).astype(np.float16)
    B = x.shape[0]
    xf = x.reshape(B, C, HW)
    wmid, wbig = _make_packs(inputs)
    in_maps = []
    for core in range(N_CORES):
        b, s = core // 2, core % 2
        xb = xf[b]
        if s == 1:
            xb = np.concatenate([xb[:, NQ:], xb[:, :NQ]], axis=1)
        in_maps.append({
            "x": np.ascontiguousarray(xb),
            "wmid": wmid, "wbig": wbig,
        })
    return in_maps


def kernel(**inputs):
    from concourse.bass_utils import run_bass_kernel_spmd

    nc = _get_nc()
    in_maps = _make_in_maps(inputs)
    res = run_bass_kernel_spmd(nc, in_maps, list(range(N_CORES)))
    x = np.asarray(inputs["x"], dtype=np.float32)
    B, _, H, W = x.shape
    out = np.empty((B, C, HW), np.float32)
    for core in range(N_CORES):
        b, s = core // 2, core % 2
        out[b, :, s * NQ:(s + 1) * NQ] = res.results[core]["y"].astype(np.float32)
    return out.reshape(B, C, H, W)
